# revision 1
# baseline (speedup 1.0000x reference)
"""Trainium2 Bass kernel for nn_DDH_49246095016535 (dense CNN + LC + FC + fuse).

Strategy: pure data parallelism over 8 NeuronCores (32 samples each).
Training-mode BN statistics are made exact via a per-layer AllGather of
per-channel partial (sum, sumsq) followed by a local combine on every core.
Convs run as PE-tile-packed matmuls ((kw, cin) on the contraction dim, kh
accumulated in PSUM); maxpool commutes with the per-channel affine BN + ReLU
(scale >= 0 here), so pooling runs on raw conv outputs and BN+ReLU is applied
once on the pooled tensor. The FC layer is decomposed per spatial position so
conv outputs feed the TensorEngine without any transposes.
"""
import sys

sys.path.insert(0, '/opt/trn_rl_repo')

import numpy as np
import ml_dtypes

import concourse.bass as bass
import concourse.tile as tile
import concourse.mybir as mybir

F32 = mybir.dt.float32
BF16 = mybir.dt.bfloat16
NPBF16 = ml_dtypes.bfloat16

N_CORES = 8
BL = 32          # samples per core
EPS = 1e-5

N1 = 256 * 62 * 62
N2 = 256 * 30 * 30
N3 = 256 * 14 * 14
N4 = 256 * 36
N5 = 256

AF = mybir.ActivationFunctionType
ALU = mybir.AluOpType
AX = mybir.AxisListType

MAX_DRAIN_WAITS = 1


def _patched_drain_and_barrier(self, tick_clock, wait_clock):
    from concourse.vector_clock import ScopedClock
    nc = self.nc
    drain_inst = nc.sync.drain()
    wait_clock.add_sem_waits(drain_inst.ins, ScopedClock({None: tick_clock.global_clock}))
    si = drain_inst.ins.sync_info
    if si is not None and len(si.on_wait) > MAX_DRAIN_WAITS:
        waits = list(si.on_wait)
        drain_inst.ins.sync_info = mybir.SyncInfo(
            on_wait=waits[:MAX_DRAIN_WAITS], on_update=list(si.on_update))
        for k in range(MAX_DRAIN_WAITS, len(waits), MAX_DRAIN_WAITS):
            extra = nc.sync.drain()
            extra.ins.sync_info = mybir.SyncInfo(
                on_wait=waits[k:k + MAX_DRAIN_WAITS], on_update=[])
    nc.all_engine_barrier()
    assert self.sems is not None
    popped = nc._tile_sem_poison_stack.pop()
    assert popped is self._sem_poison
    nc.clear_and_free_semaphores(list(self.sems.allocated().values()))
    nc.all_engine_barrier()


tile.TileContext._drain_and_barrier = _patched_drain_and_barrier


def _split_excess_waits(nc, limit=1):
    """The neuronxcc walrus codegen accepts at most one sync-wait per
    instruction; Tile's wait assigner can attach several. Move the excess
    onto same-engine NoOps inserted immediately before the instruction."""
    nid = 0
    for f in nc.m.functions:
        for b in f.blocks:
            insts = b.instructions
            new_list = []
            changed = False
            for inst in insts:
                si = getattr(inst, "sync_info", None)
                if si is not None and len(si.on_wait) > limit and inst.engine is not None:
                    waits = list(si.on_wait)
                    keep, excess = waits[:limit], waits[limit:]
                    inst.sync_info = mybir.SyncInfo(
                        on_wait=keep, on_update=list(si.on_update))
                    for k in range(0, len(excess), limit):
                        nop = mybir.InstNoOp(name=f"I-wsplit-{nid}", ins=[], outs=[])
                        nid += 1
                        nop.engine = inst.engine
                        nop.sync_info = mybir.SyncInfo(
                            on_wait=excess[k:k + limit], on_update=[])
                        new_list.append(nop)
                    changed = True
                new_list.append(inst)
            if changed:
                insts[:] = new_list
    return nc


def _stats_allgather(nc, pool, name, part_n, s1, s2, fold_groups):
    """Per-partition partial (S1,S2) -> AllGather over 8 cores -> global sums
    broadcast back into every fold-group's partition rows.
    Returns gstat [128, 2] f32."""
    st = pool.tile([128, 2], F32, name=f"st_{name}")
    nc.vector.tensor_copy(out=st[:, 0:1], in_=s1)
    nc.vector.tensor_copy(out=st[:, 1:2], in_=s2)
    G = len(fold_groups)
    cc_in = nc.dram_tensor(f"cc_{name}_in", [G, part_n, 2], F32)
    cc_out = nc.dram_tensor(f"cc_{name}_out", [N_CORES, G, part_n, 2], F32,
                            addr_space="Shared")
    for k, base in enumerate(fold_groups):
        nc.sync.dma_start(cc_in[k], st[base:base + part_n])
    nc.gpsimd.collective_compute(
        "AllGather", ALU.bypass,
        replica_groups=[list(range(N_CORES))],
        ins=[cc_in[:]], outs=[cc_out[:]],
    )
    # gall[c, s, r, g] <- cc_out[r, g, c, s], replicated into every group base
    gall = pool.tile([128, 2, N_CORES, G], F32, name=f"gall_{name}")
    src = bass.AP(tensor=cc_out, offset=0,
                  ap=[[2, part_n], [1, 2], [G * part_n * 2, N_CORES],
                      [part_n * 2, G]])
    for base in fold_groups:
        nc.sync.dma_start(gall[base:base + part_n], src)
    gstat = pool.tile([128, 2], F32, name=f"gstat_{name}")
    nc.vector.reduce_sum(gstat[:], gall[:], axis=AX.XY)
    return gstat


def _bn_scale_shift(nc, pool, name, gstat, bnp, n, eps_t):
    """gstat [128,2] raw (S1,S2); bnp [128,2] (gamma, beta).
    Returns (scale [128,1], shift [128,1]) f32."""
    mean = pool.tile([128, 1], F32, name=f"mean_{name}")
    var = pool.tile([128, 1], F32, name=f"var_{name}")
    tmp = pool.tile([128, 1], F32, name=f"tmp_{name}")
    scale = pool.tile([128, 1], F32, name=f"scale_{name}")
    shift = pool.tile([128, 1], F32, name=f"shift_{name}")
    inv_n = 1.0 / n
    nc.vector.tensor_scalar_mul(mean[:], gstat[:, 0:1], inv_n)
    nc.vector.tensor_scalar_mul(var[:], gstat[:, 1:2], inv_n)
    nc.vector.tensor_mul(tmp[:], mean[:], mean[:])
    nc.vector.tensor_sub(var[:], var[:], tmp[:])
    nc.scalar.activation(out=tmp[:], in_=var[:], func=AF.Sqrt,
                         bias=eps_t[:], scale=1.0)
    nc.vector.reciprocal(out=tmp[:], in_=tmp[:])
    nc.vector.tensor_mul(scale[:], bnp[:, 0:1], tmp[:])
    nc.vector.tensor_mul(tmp[:], mean[:], scale[:])
    nc.vector.tensor_sub(shift[:], bnp[:, 1:2], tmp[:])
    return scale, shift



def _open_pool(tc, **kw):
    cm = tc.tile_pool(**kw)
    return cm, cm.__enter__()


def build_nc():
    nc = bass.Bass("TRN2", num_devices=N_CORES)

    im1_d = nc.dram_tensor("im1", [4, 9, 8, 64, 62], BF16, kind="ExternalInput")
    w1_d = nc.dram_tensor("w1t", [9, 3, 20], BF16, kind="ExternalInput")
    w2_d = nc.dram_tensor("w2t", [40, 2, 40], BF16, kind="ExternalInput")
    w3_d = nc.dram_tensor("w3t", [80, 2, 60], BF16, kind="ExternalInput")
    lcw_d = nc.dram_tensor("lcwt", [120, 36, 2, 80], BF16, kind="ExternalInput")
    lcb_d = nc.dram_tensor("lcb", [80, 36], F32, kind="ExternalInput")
    fw1a_d = nc.dram_tensor("fcw1a", [60, 25, 768], BF16, kind="ExternalInput")
    fw1b_d = nc.dram_tensor("fcw1b", [60, 24, 768], BF16, kind="ExternalInput")
    fw2a_d = nc.dram_tensor("fcw2a", [40, 36, 768], BF16, kind="ExternalInput")
    fw2b_d = nc.dram_tensor("fcw2b", [40, 36, 768], BF16, kind="ExternalInput")
    b1_d = nc.dram_tensor("b1v", [128, 1], F32, kind="ExternalInput")
    b2_d = nc.dram_tensor("b2v", [128, 1], F32, kind="ExternalInput")
    b3_d = nc.dram_tensor("b3v", [128, 1], F32, kind="ExternalInput")
    bnp1_d = nc.dram_tensor("bnp1", [128, 2], F32, kind="ExternalInput")
    bnp2_d = nc.dram_tensor("bnp2", [128, 2], F32, kind="ExternalInput")
    bnp3_d = nc.dram_tensor("bnp3", [128, 2], F32, kind="ExternalInput")
    bnp4_d = nc.dram_tensor("bnp4", [128, 2], F32, kind="ExternalInput")
    fcb4_d = nc.dram_tensor("fcb4", [4, 192], F32, kind="ExternalInput")
    bn5p_d = nc.dram_tensor("bn5p", [4, 2, 192], F32, kind="ExternalInput")
    fw4_d = nc.dram_tensor("fw4", [128, 192], F32, kind="ExternalInput")
    fb4_d = nc.dram_tensor("fb4", [128, 12], F32, kind="ExternalInput")
    ones4_d = nc.dram_tensor("ones4", [128, 4], F32, kind="ExternalInput")
    out_d = nc.dram_tensor("out", [BL, 48], F32, kind="ExternalOutput")

    with tile.TileContext(nc) as tc:
        persist_cm, persist = _open_pool(tc, name="persist", bufs=1)
        chunks_cm, chunks = _open_pool(tc, name="chunks", bufs=6)
        psum_conv_cm, psum_conv = _open_pool(tc, name="psconv", bufs=2, space="PSUM")

        # ---------------- persistent params ----------------
        eps_t = persist.tile([128, 1], F32)
        nc.vector.memset(eps_t[:], EPS)
        b1v = persist.tile([128, 1], F32)
        nc.sync.dma_start(b1v[:], b1_d[:])
        b2v = persist.tile([128, 1], F32)
        nc.sync.dma_start(b2v[:], b2_d[:])
        b3v = persist.tile([128, 1], F32)
        nc.sync.dma_start(b3v[:], b3_d[:])
        bnp1 = persist.tile([128, 2], F32)
        nc.sync.dma_start(bnp1[:], bnp1_d[:])
        bnp2 = persist.tile([128, 2], F32)
        nc.sync.dma_start(bnp2[:], bnp2_d[:])
        bnp3 = persist.tile([128, 2], F32)
        nc.sync.dma_start(bnp3[:], bnp3_d[:])
        bnp4 = persist.tile([128, 2], F32)
        nc.sync.dma_start(bnp4[:], bnp4_d[:])
        lcb = persist.tile([80, 36], F32)
        nc.sync.dma_start(lcb[:], lcb_d[:])
        fcb4 = persist.tile([4, 192], F32)
        nc.sync.dma_start(fcb4[:], fcb4_d[:])
        bn5p = persist.tile([4, 2, 192], F32)
        nc.sync.dma_start(bn5p[:], bn5p_d[:])
        fw4 = persist.tile([128, 192], F32)
        nc.sync.dma_start(fw4[:], fw4_d[:])
        fb4 = persist.tile([128, 12], F32)
        nc.sync.dma_start(fb4[:], fb4_d[:])
        ones4 = persist.tile([128, 4], F32)
        nc.sync.dma_start(ones4[:], ones4_d[:])

        w1t = persist.tile([128, 3, 20], BF16)
        for g in range(4):
            nc.sync.dma_start(w1t[32 * g:32 * g + 9], w1_d[:])
        w2t = persist.tile([128, 2, 40], BF16)
        nc.sync.dma_start(w2t[0:40], w2_d[:])
        nc.sync.dma_start(w2t[64:104], w2_d[:])
        w3t = persist.tile([128, 2, 60], BF16)
        nc.sync.dma_start(w3t[0:80], w3_d[:])

        # persistent activations / stats
        pooled2 = persist.tile([128, 2, 8, 15, 15], BF16)   # p=64ct+m, rg, bsub
        pooled3 = persist.tile([128, 8, 2, 7, 7], BF16)     # p=64g2+o, w, b01
        h3c = persist.tile([128, 32, 49], BF16)             # c rows 0-59 & 64-123
        h3r = persist.tile([128, 32, 7, 6], BF16)            # rows dw*60+c
        lc_raw = persist.tile([128, 36, BL], BF16)           # rows o<80
        lc_bn = persist.tile([128, 36, BL], BF16)
        lc_sq = persist.tile([128, 36, BL], BF16)
        s1a_1 = persist.tile([128, 64], F32)
        s2a_1 = persist.tile([128, 64], F32)
        s1a_2 = persist.tile([128, 32], F32)
        s2a_2 = persist.tile([128, 32], F32)
        s1a_3 = persist.tile([128, 8], F32)
        s2a_3 = persist.tile([128, 8], F32)
        s1f = persist.tile([128, 1], F32)
        s2f = persist.tile([128, 1], F32)

        lcw_cm, lcw_pool = _open_pool(tc, name="lcwpool", bufs=1)
        lcw = lcw_pool.tile([128, 36, 2, 80], BF16)
        nc.sync.dma_start(lcw[0:120], lcw_d[:])

        # ================= conv1 =================
        pool1_cm, pool1_pool = _open_pool(tc, name="pool1pool", bufs=1, side="right")
        pooled1 = pool1_pool.tile([128, 4, 2, 31, 31], BF16)  # p=32j+c, g, b01

        im1_cm, im1_pool = _open_pool(tc, name="im1pool", bufs=1, side="right")
        im1 = im1_pool.tile([128, 8, 64, 62], BF16)
        for g in range(4):
            nc.sync.dma_start(im1[32 * g:32 * g + 9], im1_d[g])

        for b01 in range(2):
            for blk in range(8):
                w_idx = b01 * 8 + blk
                rows = 8 if blk < 7 else 6
                n_free = rows * 62
                banks = [psum_conv.tile([128, 496], F32, tag=f"pb{i}",
                                        name=f"c1b{i}_{w_idx}") for i in range(4)]
                for i in range(4):
                    for j in range(4):
                        b = 2 * j + b01
                        for s in range(3):
                            nc.tensor.matmul(
                                banks[i][32 * j:32 * j + 20, :n_free],
                                lhsT=w1t[32 * i:32 * i + 9, s, :],
                                rhs=im1[32 * i:32 * i + 9, b,
                                        blk * 8 + s:blk * 8 + s + rows, :],
                                start=(s == 0), stop=(s == 2),
                                tile_position=(32 * i, 32 * j),
                            )
                for i in range(4):
                    ch = w_idx * 4 + i
                    ych = chunks.tile([128, 8, 62], BF16, tag="ych",
                                      name=f"y1ch_{ch}")
                    nc.scalar.activation(
                        out=ych[:, :rows, :],
                        in_=banks[i][:, :n_free].rearrange(
                            "p (a b) -> p a b", a=rows),
                        func=AF.Identity, bias=b1v[:], scale=1.0,
                        accum_out=s1a_1[:, ch:ch + 1])
                    sq = chunks.tile([128, 8, 62], BF16, tag="ysq",
                                     name=f"y1sq_{ch}")
                    nc.vector.tensor_mul(sq[:, :rows, :], ych[:, :rows, :],
                                         ych[:, :rows, :])
                    nc.vector.reduce_sum(s2a_1[:, ch:ch + 1], sq[:, :rows, :],
                                         axis=AX.XY)
                    p1 = chunks.tile([128, 8, 31], BF16, tag="yp1",
                                     name=f"y1p1_{ch}")
                    nc.vector.tensor_max(
                        out=p1[:, :rows, :],
                        in0=ych[:, :rows, 0:62:2], in1=ych[:, :rows, 1:62:2])
                    nc.vector.tensor_max(
                        out=pooled1[:, i, b01, blk * 4:blk * 4 + rows // 2, :],
                        in0=p1[:, 0:rows:2, :], in1=p1[:, 1:rows:2, :])

        nc.vector.reduce_sum(s1f[:], s1a_1[:], axis=AX.X)
        nc.vector.reduce_sum(s2f[:], s2a_1[:], axis=AX.X)
        gstat1 = _stats_allgather(nc, persist, "bn1", 20, s1f[:], s2f[:],
                                  fold_groups=[0, 32, 64, 96])
        sc1, sh1 = _bn_scale_shift(nc, persist, "bn1", gstat1, bnp1, N1, eps_t)
        nc.scalar.activation(out=pooled1[:], in_=pooled1[:], func=AF.Relu,
                             bias=sh1[:], scale=sc1[:])

        # free im1 space; begin FC x1 weight load into it
        im1_cm.__exit__(None, None, None)
        fcw1_cm, fcw1_pool = _open_pool(tc, name="fcw1pool", bufs=1)
        fcw1 = fcw1_pool.tile([128, 25, 768], BF16)
        nc.sync.dma_start(fcw1[0:60, 0:25], fw1a_d[:])
        nc.sync.dma_start(fcw1[64:124, 0:24], fw1b_d[:])

        # ================= conv2 =================
        im2_cm, im2_pool = _open_pool(tc, name="im2pool", bufs=1)
        im2 = im2_pool.tile([128, 16, 31, 30], BF16)
        for rg in range(2):
            for dw in range(2):
                for g in (2 * rg, 2 * rg + 1):
                    for j in range(4):
                        b0 = 8 * g + 2 * j - 16 * rg
                        nc.sync.dma_start(
                            im2[64 * rg + 20 * dw:64 * rg + 20 * dw + 20,
                                b0:b0 + 2, :, :],
                            pooled1[32 * j:32 * j + 20, g, :, :, dw:dw + 30])
        pool1_cm.__exit__(None, None, None)

        for bsub in range(8):
            for h in range(2):
                w_idx = bsub * 2 + h
                rows = 16 if h == 0 else 14
                n_free = rows * 30
                banks = [psum_conv.tile([128, 480], F32, tag=f"pb{i}",
                                        name=f"c2b{i}_{w_idx}") for i in range(2)]
                for rg in range(2):
                    for ct in range(2):
                        b = 8 * ct + bsub
                        for s in range(2):
                            nc.tensor.matmul(
                                banks[rg][64 * ct:64 * ct + 40, :n_free],
                                lhsT=w2t[64 * rg:64 * rg + 40, s, :],
                                rhs=im2[64 * rg:64 * rg + 40, b,
                                        h * 16 + s:h * 16 + s + rows, :],
                                start=(s == 0), stop=(s == 1),
                                tile_position=(64 * rg, 64 * ct),
                            )
                for rg in range(2):
                    ch = w_idx * 2 + rg
                    ych = chunks.tile([128, 16, 30], BF16, tag="ych",
                                      name=f"y2ch_{ch}")
                    nc.scalar.activation(
                        out=ych[:, :rows, :],
                        in_=banks[rg][:, :n_free].rearrange(
                            "p (a b) -> p a b", a=rows),
                        func=AF.Identity, bias=b2v[:], scale=1.0,
                        accum_out=s1a_2[:, ch:ch + 1])
                    sq = chunks.tile([128, 16, 30], BF16, tag="ysq",
                                     name=f"y2sq_{ch}")
                    nc.vector.tensor_mul(sq[:, :rows, :], ych[:, :rows, :],
                                         ych[:, :rows, :])
                    nc.vector.reduce_sum(s2a_2[:, ch:ch + 1], sq[:, :rows, :],
                                         axis=AX.XY)
                    p1 = chunks.tile([128, 16, 15], BF16, tag="yp1",
                                     name=f"y2p1_{ch}")
                    nc.vector.tensor_max(
                        out=p1[:, :rows, :],
                        in0=ych[:, :rows, 0:30:2], in1=ych[:, :rows, 1:30:2])
                    nc.vector.tensor_max(
                        out=pooled2[:, rg, bsub, h * 8:h * 8 + rows // 2, :],
                        in0=p1[:, 0:rows:2, :], in1=p1[:, 1:rows:2, :])

        im2_cm.__exit__(None, None, None)
        fcw2_cm, fcw2_pool = _open_pool(tc, name="fcw2pool", bufs=1, side="right")
        fcw2 = fcw2_pool.tile([128, 36, 768], BF16)
        nc.sync.dma_start(fcw2[0:40], fw2a_d[:])
        nc.sync.dma_start(fcw2[64:104], fw2b_d[:])
        nc.vector.reduce_sum(s1f[:], s1a_2[:], axis=AX.X)
        nc.vector.reduce_sum(s2f[:], s2a_2[:], axis=AX.X)
        gstat2 = _stats_allgather(nc, persist, "bn2", 40, s1f[:], s2f[:],
                                  fold_groups=[0, 64])
        sc2, sh2 = _bn_scale_shift(nc, persist, "bn2", gstat2, bnp2, N2, eps_t)
        nc.scalar.activation(out=pooled2[:], in_=pooled2[:], func=AF.Relu,
                             bias=sh2[:], scale=sc2[:])

        # ================= conv3 =================
        im3_cm, im3_pool = _open_pool(tc, name="im3pool", bufs=1)
        im3 = im3_pool.tile([128, 32, 15, 14], BF16)
        for dw in range(2):
            for g2 in range(2):
                for rg in range(2):
                    b0 = 16 * rg + 8 * g2
                    nc.sync.dma_start(
                        im3[40 * dw:40 * dw + 40, b0:b0 + 8, :, :],
                        pooled2[64 * g2:64 * g2 + 40, rg, :, :, dw:dw + 14])

        for b2 in range(8):
            w_idx = b2
            bank = psum_conv.tile([128, 392], F32, tag="pb0", name=f"c3b_{w_idx}")
            for ct in range(2):
                b = 16 * ct + 2 * b2
                for s in range(2):
                    nc.tensor.matmul(
                        bank[64 * ct:64 * ct + 60, :],
                        lhsT=w3t[0:80, s, :],
                        rhs=im3[0:80, b:b + 2, s:s + 14, :],
                        start=(s == 0), stop=(s == 1),
                        tile_position=(0, 64 * ct),
                    )
            ych = chunks.tile([128, 2, 14, 14], BF16, tag="ych",
                              name=f"y3ch_{w_idx}")
            nc.scalar.activation(
                out=ych[:],
                in_=bank[:].rearrange("p (a b c) -> p a b c", a=2, b=14),
                func=AF.Identity, bias=b3v[:], scale=1.0,
                accum_out=s1a_3[:, w_idx:w_idx + 1])
            sq = chunks.tile([128, 2, 14, 14], BF16, tag="ysq",
                             name=f"y3sq_{w_idx}")
            nc.vector.tensor_mul(sq[:], ych[:], ych[:])
            nc.vector.reduce_sum(s2a_3[:, w_idx:w_idx + 1], sq[:], axis=AX.XYZ)
            p1 = chunks.tile([128, 2, 14, 7], BF16, tag="yp1",
                             name=f"y3p1_{w_idx}")
            nc.vector.tensor_max(out=p1[:], in0=ych[:, :, :, 0:14:2],
                                 in1=ych[:, :, :, 1:14:2])
            nc.vector.tensor_max(
                out=pooled3[:, w_idx],
                in0=p1[:, :, 0:14:2, :], in1=p1[:, :, 1:14:2, :])

        nc.vector.reduce_sum(s1f[:], s1a_3[:], axis=AX.X)
        nc.vector.reduce_sum(s2f[:], s2a_3[:], axis=AX.X)
        gstat3 = _stats_allgather(nc, persist, "bn3", 60, s1f[:], s2f[:],
                                  fold_groups=[0, 64])
        sc3, sh3 = _bn_scale_shift(nc, persist, "bn3", gstat3, bnp3, N3, eps_t)
        nc.scalar.activation(out=pooled3[:], in_=pooled3[:], func=AF.Relu,
                             bias=sh3[:], scale=sc3[:])

        # consolidate conv3 output for FC / LC
        for g2 in range(2):
            nc.sync.dma_start(
                h3c[0:60, 16 * g2:16 * g2 + 16, :],
                pooled3[64 * g2:64 * g2 + 60].rearrange(
                    "p a b c d -> p (a b) (c d)"))
        nc.sync.dma_start(h3c[64:124], h3c[0:60])
        for dw in range(2):
            nc.sync.dma_start(
                h3r[60 * dw:60 * dw + 60],
                h3c[0:60].rearrange("p b (i j) -> p b i j", i=7)[:, :, :, dw:dw + 6])

        # ================= LC layer =================
        for w4 in range(9):
            bank = psum_conv.tile([128, 4, BL], F32, tag="pb1", name=f"lcb_{w4}")
            for p4 in range(4):
                pos = w4 * 4 + p4
                i, j = divmod(pos, 6)
                for s in range(2):
                    nc.tensor.matmul(
                        bank[0:80, p4, :],
                        lhsT=lcw[0:120, pos, s, :],
                        rhs=h3r[0:120, :, i + s, j],
                        start=(s == 0), stop=(s == 1),
                        tile_position=(0, 0),
                    )
            nc.vector.scalar_tensor_tensor(
                out=lc_raw[0:80, w4 * 4:w4 * 4 + 4, :],
                in0=bank[0:80], scalar=1.0,
                in1=lcb[0:80, w4 * 4:w4 * 4 + 4, None].to_broadcast((80, 4, BL)),
                op0=ALU.mult, op1=ALU.add)

        s1_4 = persist.tile([128, 1], F32)
        s2_4 = persist.tile([128, 1], F32)
        nc.vector.reduce_sum(s1_4[0:80], lc_raw[0:80], axis=AX.XY)
        nc.vector.tensor_mul(lc_sq[0:80], lc_raw[0:80], lc_raw[0:80])
        nc.vector.reduce_sum(s2_4[0:80], lc_sq[0:80], axis=AX.XY)
        gstat4 = _stats_allgather(nc, persist, "bn4", 80,
                                  s1_4[:], s2_4[:], fold_groups=[0])
        sc4, sh4 = _bn_scale_shift(nc, persist, "bn4", gstat4, bnp4, N4, eps_t)
        nc.scalar.activation(out=lc_bn[0:80], in_=lc_raw[0:80], func=AF.Relu,
                             bias=sh4[0:80], scale=sc4[0:80])
        nc.sync.dma_start(lc_bn[64:104, :, :], lc_bn[40:80, :, :])

        # ================= FC =================
        im3_cm.__exit__(None, None, None)
        psum_conv_cm.__exit__(None, None, None)
        psum_fc_cm, psum_fc = _open_pool(tc, name="psfc", bufs=1, space="PSUM")
        acc0 = psum_fc.tile([128, 192], F32, name="fc_acc0")
        acc1 = psum_fc.tile([128, 192], F32, name="fc_acc1")
        accs = [acc0, acc1]
        for par in range(2):
            rt = 64 * par
            nt = 25 if par == 0 else 24
            for t in range(nt):
                ij = 2 * t + par
                for ct in range(4):
                    nc.tensor.matmul(
                        accs[par][32 * ct:32 * ct + 32, :],
                        lhsT=h3c[rt:rt + 60, :, ij],
                        rhs=fcw1[rt:rt + 60, t, 192 * ct:192 * ct + 192],
                        start=(t == 0), stop=False,
                        tile_position=(rt, 32 * ct),
                    )
        for kc in range(2):
            rt = 64 * kc
            for ij in range(36):
                for ct in range(4):
                    nc.tensor.matmul(
                        accs[kc][32 * ct:32 * ct + 32, :],
                        lhsT=lc_bn[rt:rt + 40, ij, :],
                        rhs=fcw2[rt:rt + 40, ij, 192 * ct:192 * ct + 192],
                        start=False, stop=(ij == 35),
                        tile_position=(rt, 32 * ct),
                    )
        # fold the two row-group accumulators; y5 holds pre-bias fc output
        t1 = persist.tile([128, 192], F32)
        nc.scalar.activation(out=t1[:], in_=acc1[:], func=AF.Copy)
        y5 = persist.tile([128, 192], F32)
        nc.vector.tensor_add(out=y5[:], in0=acc0[:], in1=t1[:])

        # bn5 stats via block-ones matmul (on pre-bias y; bias folded later)
        s5 = psum_fc.tile([4, 192], F32, name="s5")
        nc.tensor.matmul(s5[:], lhsT=ones4[:], rhs=y5[:], start=True, stop=True)
        y5q = persist.tile([128, 192], F32)
        nc.scalar.activation(out=y5q[:], in_=y5[:], func=AF.Square)
        s5q = psum_fc.tile([4, 192], F32, name="s5q")
        nc.tensor.matmul(s5q[:], lhsT=ones4[:], rhs=y5q[:], start=True, stop=True)
        st5 = persist.tile([4, 2, 192], F32)
        nc.scalar.activation(out=st5[:, 0, :], in_=s5[:], func=AF.Copy)
        nc.scalar.activation(out=st5[:, 1, :], in_=s5q[:], func=AF.Copy)

        cc5_in = nc.dram_tensor("cc_bn5_in", [4, 2, 192], F32)
        cc5_out = nc.dram_tensor("cc_bn5_out", [N_CORES, 4, 2, 192], F32,
                                 addr_space="Shared")
        nc.sync.dma_start(cc5_in[:], st5[:])
        nc.gpsimd.collective_compute(
            "AllGather", ALU.bypass,
            replica_groups=[list(range(N_CORES))],
            ins=[cc5_in[:]], outs=[cc5_out[:]],
        )
        g5all = persist.tile([4, 2, 192, N_CORES], F32)
        nc.sync.dma_start(g5all[:], cc5_out.rearrange("r g s f -> g s f r"))
        g5 = persist.tile([4, 2, 192], F32)
        nc.vector.reduce_sum(g5[:], g5all[:], axis=AX.X)

        # fold fc bias c into stats: S1' = S1 + 256c ; S2' = S2 + c*(2*S1 + 256c)
        s1p = persist.tile([4, 192], F32)
        nc.vector.scalar_tensor_tensor(
            out=s1p[:], in0=fcb4[:], scalar=256.0, in1=g5[:, 0, :],
            op0=ALU.mult, op1=ALU.add)
        t5a = persist.tile([4, 192], F32)
        nc.vector.tensor_add(out=t5a[:], in0=g5[:, 0, :], in1=s1p[:])
        t5b = persist.tile([4, 192], F32)
        nc.vector.tensor_mul(t5b[:], fcb4[:], t5a[:])
        s2p = persist.tile([4, 192], F32)
        nc.vector.tensor_add(out=s2p[:], in0=g5[:, 1, :], in1=t5b[:])

        mean5 = persist.tile([4, 192], F32)
        var5 = persist.tile([4, 192], F32)
        tmp5 = persist.tile([4, 192], F32)
        nc.vector.tensor_scalar_mul(mean5[:], s1p[:], 1.0 / N5)
        nc.vector.tensor_scalar_mul(var5[:], s2p[:], 1.0 / N5)
        nc.vector.tensor_mul(tmp5[:], mean5[:], mean5[:])
        nc.vector.tensor_sub(var5[:], var5[:], tmp5[:])
        nc.scalar.activation(out=tmp5[:], in_=var5[:], func=AF.Sqrt,
                             bias=eps_t[0:4], scale=1.0)
        nc.vector.reciprocal(out=tmp5[:], in_=tmp5[:])
        scale5 = persist.tile([4, 192], F32)
        shift5 = persist.tile([4, 192], F32)
        nc.vector.tensor_mul(scale5[:], bn5p[:, 0, :], tmp5[:])
        nc.vector.tensor_sub(tmp5[:], fcb4[:], mean5[:])
        nc.vector.tensor_mul(tmp5[:], tmp5[:], scale5[:])
        nc.vector.tensor_add(out=shift5[:], in0=bn5p[:, 1, :], in1=tmp5[:])

        # broadcast scale5/shift5 [4,192] -> [128,192] via a DRAM bounce
        sc5_d = nc.dram_tensor("sc5_scratch", [4, 192], F32)
        sh5_d = nc.dram_tensor("sh5_scratch", [4, 192], F32)
        nc.sync.dma_start(sc5_d[:], scale5[:])
        nc.sync.dma_start(sh5_d[:], shift5[:])
        scale5b = persist.tile([128, 192], F32)
        shift5b = persist.tile([128, 192], F32)
        for g in range(4):
            src_sc = bass.AP(tensor=sc5_d, offset=g * 192, ap=[[0, 32], [1, 192]])
            src_sh = bass.AP(tensor=sh5_d, offset=g * 192, ap=[[0, 32], [1, 192]])
            nc.gpsimd.dma_start(out=scale5b[32 * g:32 * g + 32, :], in_=src_sc)
            nc.gpsimd.dma_start(out=shift5b[32 * g:32 * g + 32, :], in_=src_sh)

        # apply bn5 + relu
        yb = persist.tile([128, 192], F32)
        nc.vector.tensor_mul(yb[:], y5[:], scale5b[:])
        nc.vector.tensor_add(out=yb[:], in0=yb[:], in1=shift5b[:])
        nc.vector.tensor_scalar_max(yb[:], yb[:], 0.0)

        # fuse: out[b, 12g+hh] = sum_s yb[32g+b, 16hh+s]*fw + fb
        fm = persist.tile([128, 192], F32)
        outs = persist.tile([128, 12], F32)
        nc.vector.tensor_mul(fm[:], yb[:], fw4[:])
        nc.vector.reduce_sum(
            outs[:], fm.rearrange("p (h s) -> p h s", s=16), axis=AX.X)
        nc.vector.tensor_add(out=outs[:], in0=outs[:], in1=fb4[:])
        for g in range(4):
            nc.sync.dma_start(out_d[:, 12 * g:12 * g + 12],
                              outs[32 * g:32 * g + 32, :])

        psum_fc_cm.__exit__(None, None, None)
        fcw2_cm.__exit__(None, None, None)
        fcw1_cm.__exit__(None, None, None)
        lcw_cm.__exit__(None, None, None)
        chunks_cm.__exit__(None, None, None)
        persist_cm.__exit__(None, None, None)

    return nc


# ---------------------------------------------------------------- host side

_NC_CACHE = None


def _get_nc():
    global _NC_CACHE
    if _NC_CACHE is None:
        _NC_CACHE = _split_excess_waits(build_nc())
    return _NC_CACHE


def _host_prep_shared(conv1_w, conv1_b, bn1_g, bn1_b,
                      conv2_w, conv2_b, bn2_g, bn2_b,
                      conv3_w, conv3_b, bn3_g, bn3_b,
                      lc_w, lc_b, bn4_g, bn4_b,
                      fc_w, fc_b, bn5_g, bn5_b,
                      fuse_w, fuse_b):
    d = {}
    bf = lambda a: np.ascontiguousarray(a.astype(np.float32)).astype(NPBF16)
    f3 = lambda a: np.ascontiguousarray(a).astype(np.float32)

    # wkT[dw*C+c, s, m] = conv_w[m, c, s, dw]
    d["w1t"] = bf(conv1_w.transpose(3, 1, 2, 0).reshape(9, 3, 20))
    d["w2t"] = bf(conv2_w.transpose(3, 1, 2, 0).reshape(40, 2, 40))
    d["w3t"] = bf(conv3_w.transpose(3, 1, 2, 0).reshape(80, 2, 60))

    lcw = lc_w[0]  # [80, 60, 6, 6, 4], patch idx = 2*dh + dw
    lcwt = np.zeros((120, 36, 2, 80), np.float32)
    for dw in range(2):
        for s in range(2):
            lcwt[60 * dw:60 * dw + 60, :, s, :] = (
                lcw[:, :, :, :, 2 * s + dw].reshape(80, 60, 36)
                .transpose(1, 2, 0))
    d["lcwt"] = lcwt.astype(NPBF16)
    d["lcb"] = f3(lc_b[0].reshape(80, 36))

    fw1 = fc_w[:, :2940].reshape(768, 60, 49)
    d["fcw1a"] = bf(fw1[:, :, 0::2].transpose(1, 2, 0))
    d["fcw1b"] = bf(fw1[:, :, 1::2].transpose(1, 2, 0))
    fw2 = fc_w[:, 2940:].reshape(768, 80, 36)
    d["fcw2a"] = bf(fw2[:, 0:40].transpose(1, 2, 0))
    d["fcw2b"] = bf(fw2[:, 40:80].transpose(1, 2, 0))

    def vec128(v, n, stride):
        o = np.zeros((128, 1), np.float32)
        for j in range(128 // stride):
            o[stride * j:stride * j + n, 0] = v
        return o

    d["b1v"] = vec128(conv1_b, 20, 32)
    d["b2v"] = vec128(conv2_b, 40, 64)
    d["b3v"] = vec128(conv3_b, 60, 64)

    def bnp128(g, b, n, stride):
        o = np.zeros((128, 2), np.float32)
        for j in range(128 // stride):
            o[stride * j:stride * j + n, 0] = g
            o[stride * j:stride * j + n, 1] = b
        return o

    d["bnp1"] = bnp128(bn1_g, bn1_b, 20, 32)
    d["bnp2"] = bnp128(bn2_g, bn2_b, 40, 64)
    d["bnp3"] = bnp128(bn3_g, bn3_b, 60, 64)
    o4 = np.zeros((128, 2), np.float32)
    o4[0:80, 0] = bn4_g
    o4[0:80, 1] = bn4_b
    d["bnp4"] = o4

    d["fcb4"] = f3(fc_b.reshape(4, 192))
    d["bn5p"] = f3(np.stack([bn5_g.reshape(4, 192), bn5_b.reshape(4, 192)],
                            axis=1))
    ff = fuse_w.reshape(4, 12, 16).reshape(4, 192)
    d["fw4"] = f3(np.repeat(ff, 32, axis=0))
    d["fb4"] = f3(np.repeat(fuse_b.reshape(4, 12), 32, axis=0))
    ones = np.zeros((128, 4), np.float32)
    for g in range(4):
        ones[32 * g:32 * g + 32, g] = 1.0
    d["ones4"] = ones.astype(np.float32)
    return d


def _host_prep_im1(x_shard):
    """x_shard [32, 3, 64, 64] f32 -> im1 [4, 9, 8, 64, 62] bf16,
    im1[g, 3*dw+c, b, i, j] = x[8g+b, c, i, j+dw]."""
    xs = x_shard.reshape(4, 8, 3, 64, 64)
    im1 = np.empty((4, 9, 8, 64, 62), np.float32)
    for dw in range(3):
        im1[:, 3 * dw:3 * dw + 3] = xs[:, :, :, :, dw:dw + 62].transpose(0, 2, 1, 3, 4)
    return im1.astype(NPBF16)


def kernel(**inputs):
    from concourse.bass_utils import run_bass_kernel_spmd
    x = np.asarray(inputs["x"], np.float32)
    shared = _host_prep_shared(
        **{k: np.asarray(v, np.float32) for k, v in inputs.items() if k != "x"})
    in_maps = []
    for r in range(N_CORES):
        m = dict(shared)
        m["im1"] = _host_prep_im1(x[BL * r:BL * (r + 1)])
        in_maps.append(m)
    nc = _get_nc()
    res = run_bass_kernel_spmd(nc, in_maps, core_ids=list(range(N_CORES)))
    out = np.concatenate([res.results[r]["out"] for r in range(N_CORES)], axis=0)
    return np.ascontiguousarray(out.astype(np.float32))


if __name__ == "__main__":
    sys.path.insert(0, '/root/problem')
    import reference
    inp = {k: np.asarray(v) for k, v in reference.setup_inputs().items()}
    got = kernel(**inp)
    exp = np.asarray(reference.reference(**inp))
    err = np.abs(got - exp).max() / (np.abs(exp).max() + 1e-9)
    print("out sample got:", got[0, :5])
    print("out sample exp:", exp[0, :5])
    print("rel err:", err)



# revision 22
# speedup vs baseline: 1.2143x; 1.2143x over previous
"""Trainium2 Bass kernel for nn_DDH_49246095016535 (dense CNN + LC + FC + fuse).

Strategy: pure data parallelism over 8 NeuronCores (32 samples each).
Training-mode BN statistics are made exact via a per-layer AllGather of
per-channel partial (sum, sumsq) followed by a local combine on every core.
Convs run as PE-tile-packed matmuls ((kw, cin) on the contraction dim, kh
accumulated in PSUM); maxpool commutes with the per-channel affine BN + ReLU
(scale >= 0 here), so pooling runs on raw conv outputs and BN+ReLU is applied
once on the pooled tensor. The FC layer is decomposed per spatial position so
conv outputs feed the TensorEngine without any transposes.
"""
import sys

sys.path.insert(0, '/opt/trn_rl_repo')

import numpy as np
import ml_dtypes

import concourse.bass as bass
import concourse.tile as tile
import concourse.mybir as mybir

F32 = mybir.dt.float32
BF16 = mybir.dt.bfloat16
NPBF16 = ml_dtypes.bfloat16

N_CORES = 8
BL = 32          # samples per core
EPS = 1e-5

N1 = 256 * 62 * 62
N2 = 256 * 30 * 30
N3 = 256 * 14 * 14
N4 = 256 * 36
N5 = 256

AF = mybir.ActivationFunctionType
ALU = mybir.AluOpType
AX = mybir.AxisListType

MAX_DRAIN_WAITS = 1


def _patched_drain_and_barrier(self, tick_clock, wait_clock):
    from concourse.vector_clock import ScopedClock
    nc = self.nc
    drain_inst = nc.sync.drain()
    wait_clock.add_sem_waits(drain_inst.ins, ScopedClock({None: tick_clock.global_clock}))
    si = drain_inst.ins.sync_info
    if si is not None and len(si.on_wait) > MAX_DRAIN_WAITS:
        waits = list(si.on_wait)
        drain_inst.ins.sync_info = mybir.SyncInfo(
            on_wait=waits[:MAX_DRAIN_WAITS], on_update=list(si.on_update))
        for k in range(MAX_DRAIN_WAITS, len(waits), MAX_DRAIN_WAITS):
            extra = nc.sync.drain()
            extra.ins.sync_info = mybir.SyncInfo(
                on_wait=waits[k:k + MAX_DRAIN_WAITS], on_update=[])
    nc.all_engine_barrier()
    assert self.sems is not None
    popped = nc._tile_sem_poison_stack.pop()
    assert popped is self._sem_poison
    nc.clear_and_free_semaphores(list(self.sems.allocated().values()))
    nc.all_engine_barrier()


tile.TileContext._drain_and_barrier = _patched_drain_and_barrier


def _split_excess_waits(nc, limit=1):
    """The neuronxcc walrus codegen accepts at most one sync-wait per
    instruction; Tile's wait assigner can attach several. Move the excess
    onto same-engine NoOps inserted immediately before the instruction."""
    nid = 0
    for f in nc.m.functions:
        for b in f.blocks:
            insts = b.instructions
            new_list = []
            changed = False
            for inst in insts:
                si = getattr(inst, "sync_info", None)
                if si is not None and len(si.on_wait) > limit and inst.engine is not None:
                    waits = list(si.on_wait)
                    keep, excess = waits[:limit], waits[limit:]
                    inst.sync_info = mybir.SyncInfo(
                        on_wait=keep, on_update=list(si.on_update))
                    for k in range(0, len(excess), limit):
                        nop = mybir.InstNoOp(name=f"I-wsplit-{nid}", ins=[], outs=[])
                        nid += 1
                        nop.engine = inst.engine
                        nop.sync_info = mybir.SyncInfo(
                            on_wait=excess[k:k + limit], on_update=[])
                        new_list.append(nop)
                    changed = True
                new_list.append(inst)
            if changed:
                insts[:] = new_list
    return nc


def _stats_allreduce(nc, pool, name, part_n, s1, s2, src_groups, apply_groups):
    """Per-partition partial (S1,S2) -> AllReduce(add) over 8 cores -> global
    sums broadcast back into every apply-group's partition rows.
    src_groups: partition bases holding independent partials of the same
    channels (folded locally after the collective).
    Returns gstat [128, 2] f32 laid out per apply_groups."""
    st = pool.tile([128, 2], F32, name=f"st_{name}")
    nc.vector.tensor_copy(out=st[:, 0:1], in_=s1)
    nc.vector.tensor_copy(out=st[:, 1:2], in_=s2)
    G = len(src_groups)
    cc_in = nc.dram_tensor(f"cc_{name}_in", [G, part_n, 2], F32)
    cc_out = nc.dram_tensor(f"cc_{name}_out", [G, part_n, 2], F32)
    for k, base in enumerate(src_groups):
        nc.scalar.dma_start(cc_in[k], st[base:base + part_n])
    nc.gpsimd.collective_compute(
        "AllReduce", ALU.add,
        replica_groups=[list(range(N_CORES))],
        ins=[cc_in[:]], outs=[cc_out[:]],
    )
    # gall[c, s, g] <- cc_out[g, c, s], replicated into every apply base
    gall = pool.tile([128, 2, G], F32, name=f"gall_{name}")
    src = bass.AP(tensor=cc_out, offset=0,
                  ap=[[2, part_n], [1, 2], [part_n * 2, G]])
    for base in apply_groups:
        nc.scalar.dma_start(gall[base:base + part_n], src)
    gstat = pool.tile([128, 2], F32, name=f"gstat_{name}")
    nc.vector.reduce_sum(gstat[:], gall[:], axis=AX.X)
    return gstat, cc_out


def _bn_scale_shift(nc, pool, name, gstat, bnp, n, eps_t):
    """gstat [128,2] raw (S1,S2); bnp [128,2] (gamma, beta).
    Returns (scale [128,1], shift [128,1]) f32."""
    mean = pool.tile([128, 1], F32, name=f"mean_{name}")
    var = pool.tile([128, 1], F32, name=f"var_{name}")
    tmp = pool.tile([128, 1], F32, name=f"tmp_{name}")
    scale = pool.tile([128, 1], F32, name=f"scale_{name}")
    shift = pool.tile([128, 1], F32, name=f"shift_{name}")
    inv_n = 1.0 / n
    nc.vector.tensor_scalar_mul(mean[:], gstat[:, 0:1], inv_n)
    nc.vector.tensor_scalar_mul(var[:], gstat[:, 1:2], inv_n)
    nc.vector.tensor_mul(tmp[:], mean[:], mean[:])
    nc.vector.tensor_sub(var[:], var[:], tmp[:])
    nc.scalar.activation(out=tmp[:], in_=var[:], func=AF.Sqrt,
                         bias=eps_t[:], scale=1.0)
    nc.vector.reciprocal(out=tmp[:], in_=tmp[:])
    nc.vector.tensor_mul(scale[:], bnp[:, 0:1], tmp[:])
    nc.vector.tensor_mul(tmp[:], mean[:], scale[:])
    nc.vector.tensor_sub(shift[:], bnp[:, 1:2], tmp[:])
    return scale, shift



def _open_pool(tc, **kw):
    cm = tc.tile_pool(**kw)
    return cm, cm.__enter__()


def build_nc():
    nc = bass.Bass("TRN2", num_devices=N_CORES)

    im1_d = nc.dram_tensor("im1", [4, 9, 8, 64, 62], BF16, kind="ExternalInput")
    w1_d = nc.dram_tensor("w1t", [9, 3, 20], BF16, kind="ExternalInput")
    w2_d = nc.dram_tensor("w2t", [40, 2, 40], BF16, kind="ExternalInput")
    w3_d = nc.dram_tensor("w3t", [80, 2, 60], BF16, kind="ExternalInput")
    lcw_d = nc.dram_tensor("lcwt", [120, 36, 2, 80], BF16, kind="ExternalInput")
    lcb_d = nc.dram_tensor("lcb", [80, 36], F32, kind="ExternalInput")
    fw1a_d = nc.dram_tensor("fcw1a", [60, 25, 768], BF16, kind="ExternalInput")
    fw1b_d = nc.dram_tensor("fcw1b", [60, 24, 768], BF16, kind="ExternalInput")
    fw2a_d = nc.dram_tensor("fcw2a", [40, 36, 768], BF16, kind="ExternalInput")
    fw2b_d = nc.dram_tensor("fcw2b", [40, 36, 768], BF16, kind="ExternalInput")
    b1_d = nc.dram_tensor("b1v", [128, 1], F32, kind="ExternalInput")
    b2_d = nc.dram_tensor("b2v", [128, 1], F32, kind="ExternalInput")
    b3_d = nc.dram_tensor("b3v", [128, 1], F32, kind="ExternalInput")
    bnp1_d = nc.dram_tensor("bnp1", [128, 2], F32, kind="ExternalInput")
    bnp2_d = nc.dram_tensor("bnp2", [128, 2], F32, kind="ExternalInput")
    bnp3_d = nc.dram_tensor("bnp3", [128, 2], F32, kind="ExternalInput")
    bnp3r_d = nc.dram_tensor("bnp3r", [128, 2], F32, kind="ExternalInput")
    bnp4_d = nc.dram_tensor("bnp4", [128, 2], F32, kind="ExternalInput")
    fcb4_d = nc.dram_tensor("fcb4", [4, 192], F32, kind="ExternalInput")
    bn5p_d = nc.dram_tensor("bn5p", [4, 2, 192], F32, kind="ExternalInput")
    fw4_d = nc.dram_tensor("fw4", [128, 192], F32, kind="ExternalInput")
    fb4_d = nc.dram_tensor("fb4", [128, 12], F32, kind="ExternalInput")
    ones4_d = nc.dram_tensor("ones4", [128, 4], F32, kind="ExternalInput")
    out_d = nc.dram_tensor("out", [BL, 48], F32, kind="ExternalOutput")

    with tile.TileContext(nc) as tc:
        persist_cm, persist = _open_pool(tc, name="persist", bufs=1)
        chunks_cm, chunks = _open_pool(tc, name="chunks", bufs=6)
        psum_conv_cm, psum_conv = _open_pool(tc, name="psconv", bufs=2, space="PSUM")

        # ---------------- persistent params ----------------
        eps_t = persist.tile([128, 1], F32)
        nc.vector.memset(eps_t[:], EPS)
        b1v = persist.tile([128, 1], F32)
        nc.sync.dma_start(b1v[:], b1_d[:])
        b2v = persist.tile([128, 1], F32)
        nc.sync.dma_start(b2v[:], b2_d[:])
        b3v = persist.tile([128, 1], F32)
        nc.sync.dma_start(b3v[:], b3_d[:])
        bnp1 = persist.tile([128, 2], F32)
        nc.sync.dma_start(bnp1[:], bnp1_d[:])
        bnp2 = persist.tile([128, 2], F32)
        nc.sync.dma_start(bnp2[:], bnp2_d[:])
        bnp3 = persist.tile([128, 2], F32)
        nc.sync.dma_start(bnp3[:], bnp3_d[:])
        bnp3r = persist.tile([128, 2], F32)
        nc.sync.dma_start(bnp3r[:], bnp3r_d[:])
        bnp4 = persist.tile([128, 2], F32)
        nc.sync.dma_start(bnp4[:], bnp4_d[:])
        lcb = persist.tile([80, 36], F32)
        nc.sync.dma_start(lcb[:], lcb_d[:])
        fcb4 = persist.tile([4, 192], F32)
        nc.sync.dma_start(fcb4[:], fcb4_d[:])
        bn5p = persist.tile([4, 2, 192], F32)
        nc.sync.dma_start(bn5p[:], bn5p_d[:])
        fw4 = persist.tile([128, 192], F32)
        nc.sync.dma_start(fw4[:], fw4_d[:])
        fb4 = persist.tile([128, 12], F32)
        nc.sync.dma_start(fb4[:], fb4_d[:])
        ones4 = persist.tile([128, 4], F32)
        nc.sync.dma_start(ones4[:], ones4_d[:])

        w1t = persist.tile([128, 3, 20], BF16)
        for g in range(4):
            nc.sync.dma_start(w1t[32 * g:32 * g + 9], w1_d[:])
        w2t = persist.tile([128, 2, 40], BF16)
        nc.sync.dma_start(w2t[0:40], w2_d[:])
        nc.sync.dma_start(w2t[64:104], w2_d[:])
        w3t = persist.tile([128, 2, 60], BF16)
        nc.sync.dma_start(w3t[0:80], w3_d[:])

        # persistent activations / stats
        pooled2 = persist.tile([128, 2, 8, 15, 15], BF16)   # p=64ct+m, rg, bsub
        pooled3 = persist.tile([128, 8, 2, 7, 7], BF16)     # p=64g2+o, w, b01
        h3c = persist.tile([128, 32, 49], BF16)             # c rows 0-59 & 64-123
        h3r = persist.tile([128, 32, 7, 6], BF16)            # rows dw*60+c
        lc_raw = persist.tile([128, 36, BL], BF16)           # rows o<80
        lc_bn = persist.tile([128, 36, BL], BF16)
        lc_sq = persist.tile([128, 36, BL], BF16)
        s1a_1 = persist.tile([128, 64], F32)
        s2a_1 = persist.tile([128, 64], F32)
        s1a_2 = persist.tile([128, 32], F32)
        s2a_2 = persist.tile([128, 32], F32)
        s1a_3 = persist.tile([128, 8], F32)
        s2a_3 = persist.tile([128, 8], F32)
        s1f = persist.tile([128, 1], F32)
        s2f = persist.tile([128, 1], F32)

        lcw_cm, lcw_pool = _open_pool(tc, name="lcwpool", bufs=1)
        lcw = lcw_pool.tile([128, 36, 2, 80], BF16)
        nc.sync.dma_start(lcw[0:120], lcw_d[:])

        # ================= conv1 =================
        pool1_cm, pool1_pool = _open_pool(tc, name="pool1pool", bufs=1, side="right")
        pooled1 = pool1_pool.tile([128, 4, 2, 31, 31], BF16)  # p=32j+c, g, b01

        im1_cm, im1_pool = _open_pool(tc, name="im1pool", bufs=1, side="right")
        im1 = im1_pool.tile([128, 8, 64, 62], BF16)
        # split by b-parity (b01=0 uses even pages) and spread across engine
        # DMA queues so conv1 can start after ~half the load
        ld_eng = [nc.sync, nc.gpsimd, nc.scalar]
        for par in range(2):
            for g in range(4):
                ld_eng[(par * 4 + g) % 3].dma_start(
                    im1[32 * g:32 * g + 9, par:8:2], im1_d[g, :, par:8:2])

        # early dummy AllReduce: absorbs cross-core NEFF launch skew under
        # the input DMA phase so the bn1 collective's wait is short
        bar_in = nc.dram_tensor("cc_bar_in", [1, 1], F32)
        bar_out = nc.dram_tensor("cc_bar_out", [1, 1], F32)
        nc.scalar.dma_start(bar_in[:], eps_t[0:1, :])
        nc.gpsimd.collective_compute(
            "AllReduce", ALU.add,
            replica_groups=[list(range(N_CORES))],
            ins=[bar_in[:]], outs=[bar_out[:]],
        )

        for b01 in range(2):
            for blk in range(8):
                w_idx = b01 * 8 + blk
                rows = 8 if blk < 7 else 6
                n_free = rows * 62
                banks = [psum_conv.tile([128, 496], F32, tag=f"pb{i}",
                                        name=f"c1b{i}_{w_idx}") for i in range(4)]
                for i in range(4):
                    for j in range(4):
                        b = 2 * j + b01
                        for s in range(3):
                            nc.tensor.matmul(
                                banks[i][32 * j:32 * j + 20, :n_free],
                                lhsT=w1t[32 * i:32 * i + 9, s, :],
                                rhs=im1[32 * i:32 * i + 9, b,
                                        blk * 8 + s:blk * 8 + s + rows, :],
                                start=(s == 0), stop=(s == 2),
                                tile_position=(32 * i, 32 * j),
                            )
                for i in range(4):
                    ch = w_idx * 4 + i
                    ych = chunks.tile([128, 8, 62], BF16, tag="ych",
                                      name=f"y1ch_{ch}")
                    nc.scalar.activation(
                        out=ych[:, :rows, :],
                        in_=banks[i][:, :n_free].rearrange(
                            "p (a b) -> p a b", a=rows),
                        func=AF.Identity, bias=b1v[:], scale=1.0,
                        accum_out=s1a_1[:, ch:ch + 1])
                    sq = chunks.tile([128, 8, 62], BF16, tag="ysq",
                                     name=f"y1sq_{ch}")
                    nc.vector.tensor_mul(sq[:, :rows, :], ych[:, :rows, :],
                                         ych[:, :rows, :])
                    nc.vector.reduce_sum(s2a_1[:, ch:ch + 1], sq[:, :rows, :],
                                         axis=AX.XY)
                    p1 = chunks.tile([128, 8, 31], BF16, tag="yp1",
                                     name=f"y1p1_{ch}")
                    nc.vector.tensor_max(
                        out=p1[:, :rows, :],
                        in0=ych[:, :rows, 0:62:2], in1=ych[:, :rows, 1:62:2])
                    nc.vector.tensor_max(
                        out=pooled1[:, i, b01, blk * 4:blk * 4 + rows // 2, :],
                        in0=p1[:, 0:rows:2, :], in1=p1[:, 1:rows:2, :])

        # repack RAW pooled1 -> im2 immediately (overlaps the bn1 collective);
        # BN affine+relu is applied afterwards on the packed layout
        im2_cm, im2_pool = _open_pool(tc, name="im2pool", bufs=1)
        im2 = im2_pool.tile([128, 16, 31, 30], BF16)
        rp_eng = [nc.sync, nc.gpsimd, nc.scalar]
        idx = 0
        for b01 in range(2):
            eng = ([nc.sync, nc.gpsimd, nc.scalar] if b01 == 0
                   else [nc.sync, nc.gpsimd])
            for j in range(4):
                for g in range(4):
                    rg = g // 2
                    for dw in range(2):
                        b0 = 8 * g + 2 * j - 16 * rg
                        eng[idx % len(eng)].dma_start(
                            im2[64 * rg + 20 * dw:64 * rg + 20 * dw + 20,
                                b0 + b01, :, :],
                            pooled1[32 * j:32 * j + 20, g, b01, :, dw:dw + 30])
                        idx += 1

        nc.vector.reduce_sum(s1f[:], s1a_1[:], axis=AX.X)
        nc.vector.reduce_sum(s2f[:], s2a_1[:], axis=AX.X)
        gstat1, _ = _stats_allreduce(nc, persist, "bn1", 20, s1f[:], s2f[:],
                                  src_groups=[0, 32, 64, 96],
                                  apply_groups=[0, 20, 64, 84])
        sc1, sh1 = _bn_scale_shift(nc, persist, "bn1", gstat1, bnp1, N1, eps_t)
        # affine+relu on im2 rows (64rg+20dw+c), split across engines by page
        nc.scalar.activation(out=im2[:, 0:3], in_=im2[:, 0:3], func=AF.Relu,
                             bias=sh1[:], scale=sc1[:])
        nc.scalar.activation(out=im2[:, 8:11], in_=im2[:, 8:11], func=AF.Relu,
                             bias=sh1[:], scale=sc1[:])
        for p0, p1_ in ((3, 6), (11, 14)):
            nc.vector.tensor_scalar(out=im2[:, p0:p1_], in0=im2[:, p0:p1_],
                                    scalar1=sc1[:], scalar2=sh1[:],
                                    op0=ALU.mult, op1=ALU.add)
            nc.vector.tensor_scalar_max(im2[:, p0:p1_], im2[:, p0:p1_], 0.0)
        nc.scalar.activation(out=im2[:, 6:8], in_=im2[:, 6:8], func=AF.Relu,
                             bias=sh1[:], scale=sc1[:])
        nc.vector.tensor_scalar(out=im2[:, 14:16], in0=im2[:, 14:16],
                                scalar1=sc1[:], scalar2=sh1[:],
                                op0=ALU.mult, op1=ALU.add)
        nc.vector.tensor_scalar_max(im2[:, 14:16], im2[:, 14:16], 0.0)
        # free im1 (LIFO: opened after pool1), then pool1
        im1_cm.__exit__(None, None, None)
        pool1_cm.__exit__(None, None, None)
        fcw1_cm, fcw1_pool = _open_pool(tc, name="fcw1pool", bufs=1, side="right")
        fcw1 = fcw1_pool.tile([128, 25, 768], BF16)
        nc.sync.dma_start(fcw1[0:60, 0:25], fw1a_d[:])
        nc.sync.dma_start(fcw1[64:124, 0:24], fw1b_d[:])

        # ================= conv2 =================

        for bsub in range(8):
            for h in range(2):
                w_idx = bsub * 2 + h
                rows = 16 if h == 0 else 14
                n_free = rows * 30
                banks = [psum_conv.tile([128, 480], F32, tag=f"pb{i}",
                                        name=f"c2b{i}_{w_idx}") for i in range(2)]
                for rg in range(2):
                    for ct in range(2):
                        b = 8 * ct + bsub
                        for s in range(2):
                            nc.tensor.matmul(
                                banks[rg][64 * ct:64 * ct + 40, :n_free],
                                lhsT=w2t[64 * rg:64 * rg + 40, s, :],
                                rhs=im2[64 * rg:64 * rg + 40, b,
                                        h * 16 + s:h * 16 + s + rows, :],
                                start=(s == 0), stop=(s == 1),
                                tile_position=(64 * rg, 64 * ct),
                            )
                for rg in range(2):
                    ch = w_idx * 2 + rg
                    ych = chunks.tile([128, 16, 30], BF16, tag="ych",
                                      name=f"y2ch_{ch}")
                    nc.scalar.activation(
                        out=ych[:, :rows, :],
                        in_=banks[rg][:, :n_free].rearrange(
                            "p (a b) -> p a b", a=rows),
                        func=AF.Identity, bias=b2v[:], scale=1.0,
                        accum_out=s1a_2[:, ch:ch + 1])
                    sq = chunks.tile([128, 16, 30], BF16, tag="ysq",
                                     name=f"y2sq_{ch}")
                    nc.vector.tensor_mul(sq[:, :rows, :], ych[:, :rows, :],
                                         ych[:, :rows, :])
                    nc.vector.reduce_sum(s2a_2[:, ch:ch + 1], sq[:, :rows, :],
                                         axis=AX.XY)
                    p1 = chunks.tile([128, 16, 15], BF16, tag="yp1",
                                     name=f"y2p1_{ch}")
                    nc.vector.tensor_max(
                        out=p1[:, :rows, :],
                        in0=ych[:, :rows, 0:30:2], in1=ych[:, :rows, 1:30:2])
                    nc.vector.tensor_max(
                        out=pooled2[:, rg, bsub, h * 8:h * 8 + rows // 2, :],
                        in0=p1[:, 0:rows:2, :], in1=p1[:, 1:rows:2, :])

        # repack RAW pooled2 -> im3 (overlaps the bn2 collective)
        im3_cm, im3_pool = _open_pool(tc, name="im3pool", bufs=1, side="right")
        im3 = im3_pool.tile([128, 32, 15, 14], BF16)
        idx = 0
        for bsub in range(8):
            eng = rp_eng if bsub < 6 else [nc.sync, nc.gpsimd]
            for dw in range(2):
                for g2 in range(2):
                    for rg in range(2):
                        b0 = 16 * rg + 8 * g2
                        eng[idx % len(eng)].dma_start(
                            im3[40 * dw:40 * dw + 40, b0 + bsub, :, :],
                            pooled2[64 * g2:64 * g2 + 40, rg, bsub, :,
                                    dw:dw + 14])
                        idx += 1

        im2_cm.__exit__(None, None, None)
        fcw2_cm, fcw2_pool = _open_pool(tc, name="fcw2pool", bufs=1)
        fcw2 = fcw2_pool.tile([128, 36, 768], BF16)
        nc.sync.dma_start(fcw2[0:40], fw2a_d[:])
        nc.sync.dma_start(fcw2[64:104], fw2b_d[:])
        nc.vector.reduce_sum(s1f[:], s1a_2[:], axis=AX.X)
        nc.vector.reduce_sum(s2f[:], s2a_2[:], axis=AX.X)
        gstat2, _ = _stats_allreduce(nc, persist, "bn2", 40, s1f[:], s2f[:],
                                  src_groups=[0, 64], apply_groups=[0, 40])
        sc2, sh2 = _bn_scale_shift(nc, persist, "bn2", gstat2, bnp2, N2, eps_t)
        # affine+relu on im3 rows (40dw+c), split across engines by page
        nc.scalar.activation(out=im3[0:80, 0:6], in_=im3[0:80, 0:6],
                             func=AF.Relu, bias=sh2[0:80], scale=sc2[0:80])
        nc.scalar.activation(out=im3[0:80, 16:22], in_=im3[0:80, 16:22],
                             func=AF.Relu, bias=sh2[0:80], scale=sc2[0:80])
        for p0, p1_ in ((6, 11), (22, 27)):
            nc.vector.tensor_scalar(out=im3[0:80, p0:p1_],
                                    in0=im3[0:80, p0:p1_],
                                    scalar1=sc2[0:80], scalar2=sh2[0:80],
                                    op0=ALU.mult, op1=ALU.add)
            nc.vector.tensor_scalar_max(im3[0:80, p0:p1_],
                                        im3[0:80, p0:p1_], 0.0)
        nc.scalar.activation(out=im3[0:80, 11:16], in_=im3[0:80, 11:16],
                             func=AF.Relu, bias=sh2[0:80], scale=sc2[0:80])
        nc.vector.tensor_scalar(out=im3[0:80, 27:32], in0=im3[0:80, 27:32],
                                scalar1=sc2[0:80], scalar2=sh2[0:80],
                                op0=ALU.mult, op1=ALU.add)
        nc.vector.tensor_scalar_max(im3[0:80, 27:32], im3[0:80, 27:32], 0.0)

        # ================= conv3 =================

        for b2 in range(8):
            w_idx = b2
            bank = psum_conv.tile([128, 392], F32, tag="pb0", name=f"c3b_{w_idx}")
            for ct in range(2):
                b = 16 * ct + 2 * b2
                for s in range(2):
                    nc.tensor.matmul(
                        bank[64 * ct:64 * ct + 60, :],
                        lhsT=w3t[0:80, s, :],
                        rhs=im3[0:80, b:b + 2, s:s + 14, :],
                        start=(s == 0), stop=(s == 1),
                        tile_position=(0, 64 * ct),
                    )
            ych = chunks.tile([128, 2, 14, 14], BF16, tag="ych",
                              name=f"y3ch_{w_idx}")
            nc.scalar.activation(
                out=ych[:],
                in_=bank[:].rearrange("p (a b c) -> p a b c", a=2, b=14),
                func=AF.Identity, bias=b3v[:], scale=1.0,
                accum_out=s1a_3[:, w_idx:w_idx + 1])
            sq = chunks.tile([128, 2, 14, 14], BF16, tag="ysq",
                             name=f"y3sq_{w_idx}")
            nc.vector.tensor_mul(sq[:], ych[:], ych[:])
            nc.vector.reduce_sum(s2a_3[:, w_idx:w_idx + 1], sq[:], axis=AX.XYZ)
            p1 = chunks.tile([128, 2, 14, 7], BF16, tag="yp1",
                             name=f"y3p1_{w_idx}")
            nc.vector.tensor_max(out=p1[:], in0=ych[:, :, :, 0:14:2],
                                 in1=ych[:, :, :, 1:14:2])
            nc.vector.tensor_max(
                out=pooled3[:, w_idx],
                in0=p1[:, :, 0:14:2, :], in1=p1[:, :, 1:14:2, :])

        # consolidate RAW conv3 output for FC / LC (overlaps bn3 collective)
        for g2 in range(2):
            [nc.sync, nc.gpsimd][g2].dma_start(
                h3c[0:60, 16 * g2:16 * g2 + 16, :],
                pooled3[64 * g2:64 * g2 + 60].rearrange(
                    "p a b c d -> p (a b) (c d)"))
        nc.sync.dma_start(h3c[64:124], h3c[0:60])
        for dw in range(2):
            [nc.sync, nc.gpsimd][dw].dma_start(
                h3r[60 * dw:60 * dw + 60],
                h3c[0:60].rearrange("p b (i j) -> p b i j", i=7)[:, :, :, dw:dw + 6])

        nc.vector.reduce_sum(s1f[:], s1a_3[:], axis=AX.X)
        nc.vector.reduce_sum(s2f[:], s2a_3[:], axis=AX.X)
        gstat3, cc3_out = _stats_allreduce(nc, persist, "bn3", 60, s1f[:], s2f[:],
                                  src_groups=[0, 64], apply_groups=[0, 64])
        sc3, sh3 = _bn_scale_shift(nc, persist, "bn3", gstat3, bnp3, N3, eps_t)
        gall3 = persist.tile([128, 2, 2], F32, name="gall_bn3r")
        src3 = bass.AP(tensor=cc3_out, offset=0,
                       ap=[[2, 60], [1, 2], [120, 2]])
        for base in (0, 60):
            nc.scalar.dma_start(gall3[base:base + 60], src3)
        gstat3r = persist.tile([128, 2], F32, name="gstat_bn3r")
        nc.vector.reduce_sum(gstat3r[:], gall3[:], axis=AX.X)
        sc3r, sh3r = _bn_scale_shift(nc, persist, "bn3r", gstat3r, bnp3r,
                                     N3, eps_t)
        # affine+relu on the packed layouts
        nc.scalar.activation(out=h3c[:], in_=h3c[:], func=AF.Relu,
                             bias=sh3[:], scale=sc3[:])
        nc.vector.tensor_scalar(out=h3r[0:120], in0=h3r[0:120],
                                scalar1=sc3r[0:120], scalar2=sh3r[0:120],
                                op0=ALU.mult, op1=ALU.add)
        nc.vector.tensor_scalar_max(h3r[0:120], h3r[0:120], 0.0)

        # ================= LC layer =================
        for w4 in range(9):
            bank = psum_conv.tile([128, 4, BL], F32, tag="pb1", name=f"lcb_{w4}")
            for p4 in range(4):
                pos = w4 * 4 + p4
                i, j = divmod(pos, 6)
                for s in range(2):
                    nc.tensor.matmul(
                        bank[0:80, p4, :],
                        lhsT=lcw[0:120, pos, s, :],
                        rhs=h3r[0:120, :, i + s, j],
                        start=(s == 0), stop=(s == 1),
                        tile_position=(0, 0),
                    )
            nc.vector.scalar_tensor_tensor(
                out=lc_raw[0:80, w4 * 4:w4 * 4 + 4, :],
                in0=bank[0:80], scalar=1.0,
                in1=lcb[0:80, w4 * 4:w4 * 4 + 4, None].to_broadcast((80, 4, BL)),
                op0=ALU.mult, op1=ALU.add)

        s1_4 = persist.tile([128, 1], F32)
        s2_4 = persist.tile([128, 1], F32)
        nc.vector.reduce_sum(s1_4[0:80], lc_raw[0:80], axis=AX.XY)
        nc.vector.tensor_mul(lc_sq[0:80], lc_raw[0:80], lc_raw[0:80])
        nc.vector.reduce_sum(s2_4[0:80], lc_sq[0:80], axis=AX.XY)
        gstat4, _ = _stats_allreduce(nc, persist, "bn4", 80,
                                     s1_4[:], s2_4[:], src_groups=[0],
                                     apply_groups=[0])
        sc4, sh4 = _bn_scale_shift(nc, persist, "bn4", gstat4, bnp4, N4, eps_t)
        nc.scalar.activation(out=lc_bn[0:80], in_=lc_raw[0:80], func=AF.Relu,
                             bias=sh4[0:80], scale=sc4[0:80])
        nc.sync.dma_start(lc_bn[64:104, :, :], lc_bn[40:80, :, :])

        # ================= FC =================
        im3_cm.__exit__(None, None, None)
        psum_conv_cm.__exit__(None, None, None)
        psum_fc_cm, psum_fc = _open_pool(tc, name="psfc", bufs=1, space="PSUM")
        acc0 = psum_fc.tile([128, 192], F32, name="fc_acc0")
        acc1 = psum_fc.tile([128, 192], F32, name="fc_acc1")
        accs = [acc0, acc1]
        for par in range(2):
            rt = 64 * par
            nt = 25 if par == 0 else 24
            for t in range(nt):
                ij = 2 * t + par
                for ct in range(4):
                    nc.tensor.matmul(
                        accs[par][32 * ct:32 * ct + 32, :],
                        lhsT=h3c[rt:rt + 60, :, ij],
                        rhs=fcw1[rt:rt + 60, t, 192 * ct:192 * ct + 192],
                        start=(t == 0), stop=False,
                        tile_position=(rt, 32 * ct),
                    )
        for kc in range(2):
            rt = 64 * kc
            for ij in range(36):
                for ct in range(4):
                    nc.tensor.matmul(
                        accs[kc][32 * ct:32 * ct + 32, :],
                        lhsT=lc_bn[rt:rt + 40, ij, :],
                        rhs=fcw2[rt:rt + 40, ij, 192 * ct:192 * ct + 192],
                        start=False, stop=(ij == 35),
                        tile_position=(rt, 32 * ct),
                    )
        # fold the two row-group accumulators; y5 holds pre-bias fc output
        t1 = persist.tile([128, 192], F32)
        nc.scalar.activation(out=t1[:], in_=acc1[:], func=AF.Copy)
        y5 = persist.tile([128, 192], F32)
        nc.vector.tensor_add(out=y5[:], in0=acc0[:], in1=t1[:])

        # bn5 stats via block-ones matmul (on pre-bias y; bias folded later)
        s5 = psum_fc.tile([4, 192], F32, name="s5")
        nc.tensor.matmul(s5[:], lhsT=ones4[:], rhs=y5[:], start=True, stop=True)
        y5q = persist.tile([128, 192], F32)
        nc.scalar.activation(out=y5q[:], in_=y5[:], func=AF.Square)
        s5q = psum_fc.tile([4, 192], F32, name="s5q")
        nc.tensor.matmul(s5q[:], lhsT=ones4[:], rhs=y5q[:], start=True, stop=True)
        st5 = persist.tile([4, 2, 192], F32)
        nc.scalar.activation(out=st5[:, 0, :], in_=s5[:], func=AF.Copy)
        nc.scalar.activation(out=st5[:, 1, :], in_=s5q[:], func=AF.Copy)

        cc5_in = nc.dram_tensor("cc_bn5_in", [4, 2, 192], F32)
        cc5_out = nc.dram_tensor("cc_bn5_out", [4, 2, 192], F32)
        nc.scalar.dma_start(cc5_in[:], st5[:])
        nc.gpsimd.collective_compute(
            "AllReduce", ALU.add,
            replica_groups=[list(range(N_CORES))],
            ins=[cc5_in[:]], outs=[cc5_out[:]],
        )
        g5 = persist.tile([4, 2, 192], F32)
        nc.scalar.dma_start(g5[:], cc5_out[:])

        # fold fc bias c into stats: S1' = S1 + 256c ; S2' = S2 + c*(2*S1 + 256c)
        s1p = persist.tile([4, 192], F32)
        nc.vector.scalar_tensor_tensor(
            out=s1p[:], in0=fcb4[:], scalar=256.0, in1=g5[:, 0, :],
            op0=ALU.mult, op1=ALU.add)
        t5a = persist.tile([4, 192], F32)
        nc.vector.tensor_add(out=t5a[:], in0=g5[:, 0, :], in1=s1p[:])
        t5b = persist.tile([4, 192], F32)
        nc.vector.tensor_mul(t5b[:], fcb4[:], t5a[:])
        s2p = persist.tile([4, 192], F32)
        nc.vector.tensor_add(out=s2p[:], in0=g5[:, 1, :], in1=t5b[:])

        mean5 = persist.tile([4, 192], F32)
        var5 = persist.tile([4, 192], F32)
        tmp5 = persist.tile([4, 192], F32)
        nc.vector.tensor_scalar_mul(mean5[:], s1p[:], 1.0 / N5)
        nc.vector.tensor_scalar_mul(var5[:], s2p[:], 1.0 / N5)
        nc.vector.tensor_mul(tmp5[:], mean5[:], mean5[:])
        nc.vector.tensor_sub(var5[:], var5[:], tmp5[:])
        nc.scalar.activation(out=tmp5[:], in_=var5[:], func=AF.Sqrt,
                             bias=eps_t[0:4], scale=1.0)
        nc.vector.reciprocal(out=tmp5[:], in_=tmp5[:])
        scale5 = persist.tile([4, 192], F32)
        shift5 = persist.tile([4, 192], F32)
        nc.vector.tensor_mul(scale5[:], bn5p[:, 0, :], tmp5[:])
        nc.vector.tensor_sub(tmp5[:], fcb4[:], mean5[:])
        nc.vector.tensor_mul(tmp5[:], tmp5[:], scale5[:])
        nc.vector.tensor_add(out=shift5[:], in0=bn5p[:, 1, :], in1=tmp5[:])

        # broadcast scale5/shift5 [4,192] -> [128,192] via a DRAM bounce
        sc5_d = nc.dram_tensor("sc5_scratch", [4, 192], F32)
        sh5_d = nc.dram_tensor("sh5_scratch", [4, 192], F32)
        nc.scalar.dma_start(sc5_d[:], scale5[:])
        nc.sync.dma_start(sh5_d[:], shift5[:])
        scale5b = persist.tile([128, 192], F32)
        shift5b = persist.tile([128, 192], F32)
        for g in range(4):
            src_sc = bass.AP(tensor=sc5_d, offset=g * 192, ap=[[0, 32], [1, 192]])
            src_sh = bass.AP(tensor=sh5_d, offset=g * 192, ap=[[0, 32], [1, 192]])
            nc.gpsimd.dma_start(out=scale5b[32 * g:32 * g + 32, :], in_=src_sc)
            nc.scalar.dma_start(out=shift5b[32 * g:32 * g + 32, :], in_=src_sh)

        # apply bn5 + relu
        yb = persist.tile([128, 192], F32)
        nc.vector.tensor_mul(yb[:], y5[:], scale5b[:])
        nc.vector.tensor_add(out=yb[:], in0=yb[:], in1=shift5b[:])
        nc.vector.tensor_scalar_max(yb[:], yb[:], 0.0)

        # fuse: out[b, 12g+hh] = sum_s yb[32g+b, 16hh+s]*fw + fb
        fm = persist.tile([128, 192], F32)
        outs = persist.tile([128, 12], F32)
        nc.vector.tensor_mul(fm[:], yb[:], fw4[:])
        nc.vector.reduce_sum(
            outs[:], fm.rearrange("p (h s) -> p h s", s=16), axis=AX.X)
        nc.vector.tensor_add(out=outs[:], in0=outs[:], in1=fb4[:])
        out_eng = [nc.sync, nc.gpsimd, nc.scalar, nc.sync]
        for g in range(4):
            out_eng[g].dma_start(out_d[:, 12 * g:12 * g + 12],
                                 outs[32 * g:32 * g + 32, :])

        psum_fc_cm.__exit__(None, None, None)
        fcw2_cm.__exit__(None, None, None)
        fcw1_cm.__exit__(None, None, None)
        lcw_cm.__exit__(None, None, None)
        chunks_cm.__exit__(None, None, None)
        persist_cm.__exit__(None, None, None)

    return nc


# ---------------------------------------------------------------- host side

_NC_CACHE = None


def _get_nc():
    global _NC_CACHE
    if _NC_CACHE is None:
        _NC_CACHE = _split_excess_waits(build_nc())
    return _NC_CACHE


def _host_prep_shared(conv1_w, conv1_b, bn1_g, bn1_b,
                      conv2_w, conv2_b, bn2_g, bn2_b,
                      conv3_w, conv3_b, bn3_g, bn3_b,
                      lc_w, lc_b, bn4_g, bn4_b,
                      fc_w, fc_b, bn5_g, bn5_b,
                      fuse_w, fuse_b):
    d = {}
    bf = lambda a: np.ascontiguousarray(a.astype(np.float32)).astype(NPBF16)
    f3 = lambda a: np.ascontiguousarray(a).astype(np.float32)

    # wkT[dw*C+c, s, m] = conv_w[m, c, s, dw]
    d["w1t"] = bf(conv1_w.transpose(3, 1, 2, 0).reshape(9, 3, 20))
    d["w2t"] = bf(conv2_w.transpose(3, 1, 2, 0).reshape(40, 2, 40))
    d["w3t"] = bf(conv3_w.transpose(3, 1, 2, 0).reshape(80, 2, 60))

    lcw = lc_w[0]  # [80, 60, 6, 6, 4], patch idx = 2*dh + dw
    lcwt = np.zeros((120, 36, 2, 80), np.float32)
    for dw in range(2):
        for s in range(2):
            lcwt[60 * dw:60 * dw + 60, :, s, :] = (
                lcw[:, :, :, :, 2 * s + dw].reshape(80, 60, 36)
                .transpose(1, 2, 0))
    d["lcwt"] = lcwt.astype(NPBF16)
    d["lcb"] = f3(lc_b[0].reshape(80, 36))

    fw1 = fc_w[:, :2940].reshape(768, 60, 49)
    d["fcw1a"] = bf(fw1[:, :, 0::2].transpose(1, 2, 0))
    d["fcw1b"] = bf(fw1[:, :, 1::2].transpose(1, 2, 0))
    fw2 = fc_w[:, 2940:].reshape(768, 80, 36)
    d["fcw2a"] = bf(fw2[:, 0:40].transpose(1, 2, 0))
    d["fcw2b"] = bf(fw2[:, 40:80].transpose(1, 2, 0))

    def vec128(v, n, stride):
        o = np.zeros((128, 1), np.float32)
        for j in range(128 // stride):
            o[stride * j:stride * j + n, 0] = v
        return o

    d["b1v"] = vec128(conv1_b, 20, 32)
    d["b2v"] = vec128(conv2_b, 40, 64)
    d["b3v"] = vec128(conv3_b, 60, 64)

    def bnp128(g, b, n, stride):
        o = np.zeros((128, 2), np.float32)
        for j in range(128 // stride):
            o[stride * j:stride * j + n, 0] = g
            o[stride * j:stride * j + n, 1] = b
        return o

    # bnp1: im2-apply layout, rows 64rg+20dw+c  (bases 0, 20, 64, 84)
    o1 = np.zeros((128, 2), np.float32)
    for base in (0, 20, 64, 84):
        o1[base:base + 20, 0] = bn1_g
        o1[base:base + 20, 1] = bn1_b
    d["bnp1"] = o1
    # bnp2: im3-apply layout, rows 40dw+c  (bases 0, 40)
    d["bnp2"] = bnp128(bn2_g, bn2_b, 40, 40)
    d["bnp3"] = bnp128(bn3_g, bn3_b, 60, 64)
    # bnp3r: h3r-apply layout, rows 60dw+c  (bases 0, 60)
    d["bnp3r"] = bnp128(bn3_g, bn3_b, 60, 60)
    o4 = np.zeros((128, 2), np.float32)
    o4[0:80, 0] = bn4_g
    o4[0:80, 1] = bn4_b
    d["bnp4"] = o4

    d["fcb4"] = f3(fc_b.reshape(4, 192))
    d["bn5p"] = f3(np.stack([bn5_g.reshape(4, 192), bn5_b.reshape(4, 192)],
                            axis=1))
    ff = fuse_w.reshape(4, 12, 16).reshape(4, 192)
    d["fw4"] = f3(np.repeat(ff, 32, axis=0))
    d["fb4"] = f3(np.repeat(fuse_b.reshape(4, 12), 32, axis=0))
    ones = np.zeros((128, 4), np.float32)
    for g in range(4):
        ones[32 * g:32 * g + 32, g] = 1.0
    d["ones4"] = ones.astype(np.float32)
    return d


def _host_prep_im1(x_shard):
    """x_shard [32, 3, 64, 64] f32 -> im1 [4, 9, 8, 64, 62] bf16,
    im1[g, 3*dw+c, b, i, j] = x[8g+b, c, i, j+dw]."""
    xs = x_shard.reshape(4, 8, 3, 64, 64)
    im1 = np.empty((4, 9, 8, 64, 62), np.float32)
    for dw in range(3):
        im1[:, 3 * dw:3 * dw + 3] = xs[:, :, :, :, dw:dw + 62].transpose(0, 2, 1, 3, 4)
    return im1.astype(NPBF16)


def kernel(**inputs):
    from concourse.bass_utils import run_bass_kernel_spmd
    x = np.asarray(inputs["x"], np.float32)
    shared = _host_prep_shared(
        **{k: np.asarray(v, np.float32) for k, v in inputs.items() if k != "x"})
    in_maps = []
    for r in range(N_CORES):
        m = dict(shared)
        m["im1"] = _host_prep_im1(x[BL * r:BL * (r + 1)])
        in_maps.append(m)
    nc = _get_nc()
    res = run_bass_kernel_spmd(nc, in_maps, core_ids=list(range(N_CORES)))
    out = np.concatenate([res.results[r]["out"] for r in range(N_CORES)], axis=0)
    return np.ascontiguousarray(out.astype(np.float32))


if __name__ == "__main__":
    sys.path.insert(0, '/root/problem')
    import reference
    inp = {k: np.asarray(v) for k, v in reference.setup_inputs().items()}
    got = kernel(**inp)
    exp = np.asarray(reference.reference(**inp))
    err = np.abs(got - exp).max() / (np.abs(exp).max() + 1e-9)
    print("out sample got:", got[0, :5])
    print("out sample exp:", exp[0, :5])
    print("rel err:", err)

